# revision 3
# baseline (speedup 1.0000x reference)
"""MultiHeadAttention Trainium2 Bass kernel, 8-core (batch x head-group) sharded.

Reference computation (B=4, S=2048, D=1024, H=16, d_k=64):
    Q = query @ W_q.T ; K = key @ W_k.T ; V = value @ W_v.T
    per head: attn = softmax(Q K^T / 8) @ V
    out = concat_heads(attn) @ W_o.T

Sharding: core c handles batch b = c // 2 and head-group hg = c % 2 (8 heads,
a 512-wide slice of the model dim). Host pre-transposes activations/weights so
every on-device matmul contracts along partitions; core-pair partial outputs
(row-parallel W_o) are summed on the host during unsharding.

Per-core dataflow (all matmul inputs float32r):
    K.T[d', s] = (W_k.T slice).T @ x_k.T   (pattern B, d' on partitions)
    Q.T[d', s] likewise, projected on demand per 512-wide q-block
    V[s, d']   = (x_v.T).T @ W_v.T         (pattern A, natural layout)
    S.T[k, q]  = (K_h.T).T @ Q_h.T         (two heads row-packed, K=64)
    expS.T     = exp(S.T / 8)              (ACT, groups of 3 PSUM banks)
    O.T+denom  = [V_h | 1].T @ expS.T      (M=65, accumulated over k tiles)
    O.T norm   = O.T * (1/denom)           (DVE + gpsimd partition broadcast)
    out[s, :]  = O.T.T @ W_o.T slice       (partial; host adds core pairs)
"""
import sys

sys.path.insert(0, "/opt/trn_rl_repo")

import numpy as np

import concourse.bass as bass  # noqa: F401
import concourse.tile as tile
from concourse import bacc, mybir
from concourse.bass_utils import run_bass_kernel_spmd

F32R = mybir.dt.float32r
F32 = mybir.dt.float32
EXP = mybir.ActivationFunctionType.Exp
MULT = mybir.AluOpType.mult

B, S, D = 4, 2048, 1024
H_PER_CORE = 8      # heads per core
DH = 64             # head dim
DP = 512            # per-core model-dim slice (8 heads x 64)
NT = 4              # d' tiles / head pairs per core
SB = 4              # 512-wide s/q blocks
KT = 16             # 128-wide k tiles
PKT = 8             # 128-wide contraction tiles for projections (D / 128)
VW = DH + 1         # V columns per head incl. ones column

_RUN_KWARGS = {}
_LAST_RESULT = []


def build_nc():
    nc = bacc.Bacc("TRN2", target_bir_lowering=False, debug=False)

    xqt = nc.dram_tensor("xqt", [D, S], F32R, kind="ExternalInput")
    xkt = nc.dram_tensor("xkt", [D, S], F32R, kind="ExternalInput")
    xvt = nc.dram_tensor("xvt", [D, S], F32R, kind="ExternalInput")
    wqt = nc.dram_tensor("wqt", [D, DP], F32R, kind="ExternalInput")
    wkt = nc.dram_tensor("wkt", [D, DP], F32R, kind="ExternalInput")
    wvt = nc.dram_tensor("wvt", [D, DP], F32R, kind="ExternalInput")
    wot = nc.dram_tensor("wot", [DP, D], F32R, kind="ExternalInput")
    out = nc.dram_tensor("out", [S, D], F32, kind="ExternalOutput")

    with tile.TileContext(nc) as tc:
        with tc.tile_pool(name="persist", bufs=1) as persist, \
             tc.tile_pool(name="psacc", bufs=2, space="PSUM") as psacc, \
             tc.tile_pool(name="pssc", bufs=2, space="PSUM") as pssc:

            # ---- persistent SBUF ----
            wq_s = persist.tile([128, PKT, DP], F32R)
            wot_s = persist.tile([128, NT, D], F32R)
            kt_s = persist.tile([128, NT, S], F32R)          # K.T
            vext_s = persist.tile([128, KT, H_PER_CORE * VW], F32R)  # [V_h | 1]

            nc.sync.dma_start(wq_s[:], wqt.rearrange("(t p) m -> p t m", p=128))
            nc.sync.dma_start(wot_s[:], wot.rearrange("(t p) m -> p t m", p=128))
            # ones columns for the denominator rows (V part is written below)
            ones_f = persist.tile([128, KT, H_PER_CORE], F32)
            nc.vector.memset(ones_f[:], 1.0)
            nc.vector.tensor_copy(
                vext_s[:].rearrange("p k (h c) -> p k h c", c=VW)[:, :, :, DH:DH + 1],
                ones_f[:, :, :, None],
            )

            # ============ phase 1+2: K.T and V projections ============
            with tc.tile_pool(name="proj", bufs=2) as proj:
                wk_s = proj.tile([128, PKT, DP], F32R, bufs=1)
                wv_s = proj.tile([128, PKT, DP], F32R, bufs=1)
                nc.sync.dma_start(wk_s[:], wkt.rearrange("(t p) m -> p t m", p=128))
                nc.sync.dma_start(wv_s[:], wvt.rearrange("(t p) m -> p t m", p=128))

                # K.T projection: kt_s[:, t, sb*512:] = (wk col-block).T @ xk.T
                for sb in range(SB):
                    xk_b = proj.tile([128, PKT, 512], F32R, tag="xk",
                                     name=f"xk_{sb}")
                    nc.sync.dma_start(
                        xk_b[:],
                        xkt[:, sb * 512:(sb + 1) * 512].rearrange(
                            "(t p) s -> p t s", p=128),
                    )
                    for t in range(NT):
                        ps = psacc.tile([128, 512], F32, tag="acc",
                                        name=f"psk_{sb}_{t}")
                        for kt in range(PKT):
                            nc.tensor.matmul(
                                ps[:],
                                wk_s[:, kt, t * 128:(t + 1) * 128],
                                xk_b[:, kt, :],
                                start=kt == 0, stop=kt == PKT - 1,
                            )
                        nc.vector.tensor_copy(
                            kt_s[:, t, sb * 512:(sb + 1) * 512], ps[:])

                # V projection into [V_h | 1] layout
                for st in range(KT):
                    xv_b = proj.tile([128, PKT, 128], F32R, tag="xv",
                                     name=f"xv_{st}")
                    nc.sync.dma_start(
                        xv_b[:],
                        xvt[:, st * 128:(st + 1) * 128].rearrange(
                            "(t p) s -> p t s", p=128),
                    )
                    ps = psacc.tile([128, 512], F32, tag="acc", name=f"psv_{st}")
                    for kt in range(PKT):
                        nc.tensor.matmul(
                            ps[:], xv_b[:, kt, :], wv_s[:, kt, :],
                            start=kt == 0, stop=kt == PKT - 1,
                        )
                    nc.vector.tensor_copy(
                        vext_s[:, st, :].rearrange(
                            "p (h c) -> p h c", c=VW)[:, :, 0:DH],
                        ps[:].rearrange("p (h c) -> p h c", c=DH),
                    )

            # ============ phase 3: attention + W_o, per 512-wide q block ==========
            with tc.tile_pool(name="att", bufs=2) as att:
                for qb in range(SB):
                    qsl = slice(qb * 512, (qb + 1) * 512)

                    # Q.T for this q block, all 4 d' tiles
                    xq_b = att.tile([128, PKT, 512], F32R, tag="xq", bufs=1,
                                    name=f"xq_{qb}")
                    nc.sync.dma_start(
                        xq_b[:],
                        xqt[:, qsl].rearrange("(t p) s -> p t s", p=128),
                    )
                    qt_b = att.tile([128, NT, 512], F32R, tag="qt",
                                    name=f"qt_{qb}")
                    for t in range(NT):
                        ps = psacc.tile([128, 512], F32, tag="acc",
                                        name=f"psq_{qb}_{t}")
                        for kt in range(PKT):
                            nc.tensor.matmul(
                                ps[:],
                                wq_s[:, kt, t * 128:(t + 1) * 128],
                                xq_b[:, kt, :],
                                start=kt == 0, stop=kt == PKT - 1,
                            )
                        nc.vector.tensor_copy(qt_b[:, t, :], ps[:])

                    ot_b = att.tile([128, NT, 512], F32R, tag="ot",
                                    name=f"ot_{qb}")
                    for t in range(NT):
                        # two heads: A on partitions 0:64, B on 64:128
                        ota = psacc.tile([65, 512], F32, tag="acc",
                                         name=f"ota_{qb}_{t}")
                        otb = psacc.tile([65, 512], F32, tag="acc",
                                         name=f"otb_{qb}_{t}")
                        for g0 in range(0, KT, 3):
                            gn = min(3, KT - g0)
                            sca = pssc.tile([128, 3, 512], F32, tag="sc",
                                            name=f"sca_{qb}_{t}_{g0}")
                            scb = pssc.tile([128, 3, 512], F32, tag="sc",
                                            name=f"scb_{qb}_{t}_{g0}")
                            for j in range(gn):
                                kt = g0 + j
                                ksl = slice(kt * 128, (kt + 1) * 128)
                                nc.tensor.matmul(
                                    sca[:, j, :], kt_s[0:64, t, ksl],
                                    qt_b[0:64, t, :],
                                    start=True, stop=True, tile_position=(0, 0),
                                )
                                nc.tensor.matmul(
                                    scb[:, j, :], kt_s[64:128, t, ksl],
                                    qt_b[64:128, t, :],
                                    start=True, stop=True, tile_position=(64, 0),
                                )
                            ea = att.tile([128, 3, 512], F32R, tag="exp", bufs=4,
                                          name=f"ea_{qb}_{t}_{g0}")
                            eb = att.tile([128, 3, 512], F32R, tag="exp", bufs=4,
                                          name=f"eb_{qb}_{t}_{g0}")
                            nc.scalar.activation(ea[:, 0:gn, :], sca[:, 0:gn, :],
                                                 EXP, scale=0.125)
                            nc.scalar.activation(eb[:, 0:gn, :], scb[:, 0:gn, :],
                                                 EXP, scale=0.125)
                            ha, hb = 2 * t, 2 * t + 1
                            for j in range(gn):
                                kt = g0 + j
                                nc.tensor.matmul(
                                    ota[:], vext_s[:, kt, ha * VW:(ha + 1) * VW],
                                    ea[:, j, :],
                                    start=kt == 0, stop=kt == KT - 1,
                                )
                                nc.tensor.matmul(
                                    otb[:], vext_s[:, kt, hb * VW:(hb + 1) * VW],
                                    eb[:, j, :],
                                    start=kt == 0, stop=kt == KT - 1,
                                )
                        # normalize: rows 0:64 are O.T, row 64 the denominator
                        for nm, ot_ps, psl in (("a", ota, slice(0, 64)),
                                               ("b", otb, slice(64, 128))):
                            rd = att.tile([1, 512], F32, tag="rd", bufs=2,
                                          name=f"rd{nm}_{qb}_{t}")
                            nc.vector.reciprocal(rd[:], ot_ps[64:65, :])
                            rb = att.tile([64, 512], F32, tag="rb", bufs=2,
                                          name=f"rb{nm}_{qb}_{t}")
                            nc.gpsimd.partition_broadcast(rb[:], rd[:])
                            nc.vector.tensor_tensor(
                                ot_b[psl, t, :], ot_ps[0:64, :], rb[:], MULT)

                    # ---- W_o stage for the 4 s-tiles of this q block ----
                    for si in range(4):
                        st = qb * 4 + si
                        ssl = slice(si * 128, (si + 1) * 128)
                        for dm in range(2):
                            ps = psacc.tile([128, 512], F32, tag="acc",
                                            name=f"pso_{st}_{dm}")
                            for t in range(NT):
                                nc.tensor.matmul(
                                    ps[:], ot_b[:, t, ssl],
                                    wot_s[:, t, dm * 512:(dm + 1) * 512],
                                    start=t == 0, stop=t == NT - 1,
                                )
                            ob = att.tile([128, 512], F32, tag="ob", bufs=3,
                                          name=f"ob_{st}_{dm}")
                            nc.vector.tensor_copy(ob[:], ps[:])
                            nc.sync.dma_start(
                                out[st * 128:(st + 1) * 128,
                                    dm * 512:(dm + 1) * 512],
                                ob[:])
    nc.compile()
    return nc


_NC_CACHE = []


def kernel(**inputs):
    query = np.asarray(inputs["query"], dtype=np.float32)
    key = np.asarray(inputs["key"], dtype=np.float32)
    value = np.asarray(inputs["value"], dtype=np.float32)
    w_q = np.asarray(inputs["W_q"], dtype=np.float32)
    w_k = np.asarray(inputs["W_k"], dtype=np.float32)
    w_v = np.asarray(inputs["W_v"], dtype=np.float32)
    w_o = np.asarray(inputs["W_o"], dtype=np.float32)

    in_maps = []
    for c in range(8):
        b, hg = c // 2, c % 2
        dsl = slice(hg * DP, (hg + 1) * DP)
        in_maps.append({
            "xqt": np.ascontiguousarray(query[b].T),
            "xkt": np.ascontiguousarray(key[b].T),
            "xvt": np.ascontiguousarray(value[b].T),
            "wqt": np.ascontiguousarray(w_q[dsl, :].T),
            "wkt": np.ascontiguousarray(w_k[dsl, :].T),
            "wvt": np.ascontiguousarray(w_v[dsl, :].T),
            "wot": np.ascontiguousarray(w_o[:, dsl].T),
        })

    if not _NC_CACHE:
        _NC_CACHE.append(build_nc())
    nc = _NC_CACHE[0]
    res = run_bass_kernel_spmd(nc, in_maps, core_ids=list(range(8)),
                               **_RUN_KWARGS)
    _LAST_RESULT.clear()
    _LAST_RESULT.append(res)
    parts = [r["out"] for r in res.results]
    full = np.empty((B, S, D), dtype=np.float32)
    for b in range(B):
        full[b] = parts[2 * b] + parts[2 * b + 1]
    return full


# revision 7
# speedup vs baseline: 1.1982x; 1.1982x over previous
"""MultiHeadAttention Trainium2 Bass kernel, 8-core (batch x head-group) sharded.

Reference computation (B=4, S=2048, D=1024, H=16, d_k=64):
    Q = query @ W_q.T ; K = key @ W_k.T ; V = value @ W_v.T
    per head: attn = softmax(Q K^T / 8) @ V
    out = concat_heads(attn) @ W_o.T

Sharding: core c handles batch b = c // 2 and head-group hg = c % 2 (8 heads,
a 512-wide slice of the model dim). Host pre-transposes activations/weights so
every on-device matmul contracts along partitions; core-pair partial outputs
(row-parallel W_o) are summed on the host during unsharding.

Per-core dataflow (all matmul inputs float32r):
    K.T[d', s] = (W_k.T slice).T @ x_k.T   (pattern B, d' on partitions)
    Q.T[d', s] likewise, projected on demand per 512-wide q-block
    V[s, d']   = (x_v.T).T @ W_v.T         (pattern A, natural layout)
    S.T[k, q]  = (K_h.T).T @ Q_h.T         (two heads row-packed, K=64)
    expS.T     = exp(S.T / 8)              (ACT, groups of 3 PSUM banks)
    O.T+denom  = [V_h | 1].T @ expS.T      (M=65, accumulated over k tiles)
    O.T norm   = O.T * (1/denom)           (DVE + gpsimd partition broadcast)
    out[s, :]  = O.T.T @ W_o.T slice       (partial; host adds core pairs)
"""
import sys

sys.path.insert(0, "/opt/trn_rl_repo")

import numpy as np

import concourse.bass as bass  # noqa: F401
import concourse.tile as tile
from concourse import bacc, mybir
from concourse.bass_utils import run_bass_kernel_spmd

F32R = mybir.dt.float32r
F32 = mybir.dt.float32
EXP = mybir.ActivationFunctionType.Exp
MULT = mybir.AluOpType.mult
DIV = mybir.AluOpType.divide

B, S, D = 4, 2048, 1024
H_PER_CORE = 8      # heads per core
DH = 64             # head dim
DP = 512            # per-core model-dim slice (8 heads x 64)
NT = 4              # d' tiles / head pairs per core
SB = 4              # 512-wide s/q blocks
KT = 16             # 128-wide k tiles
PKT = 8             # 128-wide contraction tiles for projections (D / 128)
VW = DH + 1         # V columns per head incl. ones column

_RUN_KWARGS = {}
_LAST_RESULT = []


def build_nc():
    nc = bacc.Bacc("TRN2", target_bir_lowering=False, debug=False)

    xqt = nc.dram_tensor("xqt", [D, S], F32R, kind="ExternalInput")
    xkt = nc.dram_tensor("xkt", [D, S], F32R, kind="ExternalInput")
    xvt = nc.dram_tensor("xvt", [D, S], F32R, kind="ExternalInput")
    wqt = nc.dram_tensor("wqt", [D, DP], F32R, kind="ExternalInput")
    wkt = nc.dram_tensor("wkt", [D, DP], F32R, kind="ExternalInput")
    wvt = nc.dram_tensor("wvt", [D, DP], F32R, kind="ExternalInput")
    wot = nc.dram_tensor("wot", [DP, D], F32R, kind="ExternalInput")
    out = nc.dram_tensor("out", [S, D], F32, kind="ExternalOutput")

    with tile.TileContext(nc) as tc:
        with tc.tile_pool(name="persist", bufs=1) as persist, \
             tc.tile_pool(name="psacc", bufs=2, space="PSUM") as psacc, \
             tc.tile_pool(name="pssc", bufs=3, space="PSUM") as pssc:

            # ---- persistent SBUF ----
            wq_s = persist.tile([128, PKT, DP], F32R)
            wot_s = persist.tile([128, NT, D], F32R)
            kt_s = persist.tile([128, NT, S], F32R)          # K.T
            vext_s = persist.tile([128, KT, H_PER_CORE * VW], F32R)  # [V_h | 1]

            nc.sync.dma_start(wq_s[:], wqt.rearrange("(t p) m -> p t m", p=128))
            nc.sync.dma_start(wot_s[:], wot.rearrange("(t p) m -> p t m", p=128))
            # ones columns for the denominator rows (V part is written below)
            ones_f = persist.tile([128, KT, H_PER_CORE], F32)
            nc.vector.memset(ones_f[:], 1.0)
            nc.vector.tensor_copy(
                vext_s[:].rearrange("p k (h c) -> p k h c", c=VW)[:, :, :, DH:DH + 1],
                ones_f[:, :, :, None],
            )

            # ============ phase 1+2: K.T and V projections ============
            with tc.tile_pool(name="proj", bufs=2) as proj:
                wk_s = proj.tile([128, PKT, DP], F32R, bufs=1)
                wv_s = proj.tile([128, PKT, DP], F32R, bufs=1)
                nc.sync.dma_start(wk_s[:], wkt.rearrange("(t p) m -> p t m", p=128))
                nc.sync.dma_start(wv_s[:], wvt.rearrange("(t p) m -> p t m", p=128))

                # V projection into [V_h | 1] layout (2 s-tiles per DMA)
                for sg in range(KT // 2):
                    xv_b = proj.tile([128, PKT, 256], F32R, tag="xv",
                                     name=f"xv_{sg}")
                    nc.sync.dma_start(
                        xv_b[:],
                        xvt[:, sg * 256:(sg + 1) * 256].rearrange(
                            "(t p) s -> p t s", p=128),
                    )
                    for half in range(2):
                        st = sg * 2 + half
                        ps = psacc.tile([128, 512], F32, tag="acc",
                                        name=f"psv_{st}")
                        for kt in range(PKT):
                            nc.tensor.matmul(
                                ps[:], xv_b[:, kt, half * 128:(half + 1) * 128],
                                wv_s[:, kt, :],
                                start=kt == 0, stop=kt == PKT - 1,
                            )
                        nc.vector.tensor_copy(
                            vext_s[:, st, :].rearrange(
                                "p (h c) -> p h c", c=VW)[:, :, 0:DH],
                            ps[:].rearrange("p (h c) -> p h c", c=DH),
                        )

                # K.T projection: kt_s[:, t, sb*512:] = (wk col-block).T @ xk.T
                for sb in range(SB):
                    xk_b = proj.tile([128, PKT, 512], F32R, tag="xk",
                                     name=f"xk_{sb}")
                    nc.sync.dma_start(
                        xk_b[:],
                        xkt[:, sb * 512:(sb + 1) * 512].rearrange(
                            "(t p) s -> p t s", p=128),
                    )
                    for t in range(NT):
                        ps = psacc.tile([128, 512], F32, tag="acc",
                                        name=f"psk_{sb}_{t}")
                        for kt in range(PKT):
                            nc.tensor.matmul(
                                ps[:],
                                wk_s[:, kt, t * 128:(t + 1) * 128],
                                xk_b[:, kt, :],
                                start=kt == 0, stop=kt == PKT - 1,
                            )
                        nc.vector.tensor_copy(
                            kt_s[:, t, sb * 512:(sb + 1) * 512], ps[:])
            # ============ phase 3: attention + W_o, per 512-wide q block ==========
            with tc.tile_pool(name="att", bufs=2) as att:
                for qb in range(SB):
                    qsl = slice(qb * 512, (qb + 1) * 512)

                    # Q.T for this q block, all 4 d' tiles
                    xq_b = att.tile([128, PKT, 512], F32R, tag="xq", bufs=1,
                                    name=f"xq_{qb}")
                    nc.sync.dma_start(
                        xq_b[:],
                        xqt[:, qsl].rearrange("(t p) s -> p t s", p=128),
                    )
                    qt_b = att.tile([128, NT, 512], F32R, tag="qt",
                                    name=f"qt_{qb}")
                    for t in range(NT):
                        ps = psacc.tile([128, 512], F32, tag="acc",
                                        name=f"psq_{qb}_{t}")
                        for kt in range(PKT):
                            nc.tensor.matmul(
                                ps[:],
                                wq_s[:, kt, t * 128:(t + 1) * 128],
                                xq_b[:, kt, :],
                                start=kt == 0, stop=kt == PKT - 1,
                            )
                        nc.vector.tensor_copy(qt_b[:, t, :], ps[:])

                    ot_b = att.tile([128, NT, 512], F32R, tag="ot",
                                    name=f"ot_{qb}")
                    for t in range(NT):
                        # two heads: A on partitions 0:64, B on 64:128
                        ota = psacc.tile([65, 512], F32, tag="acc",
                                         name=f"ota_{qb}_{t}")
                        otb = psacc.tile([65, 512], F32, tag="acc",
                                         name=f"otb_{qb}_{t}")
                        for g0 in range(0, KT, 2):
                            gn = 2
                            sca = pssc.tile([128, 2, 512], F32, tag="sc",
                                            name=f"sca_{qb}_{t}_{g0}")
                            scb = pssc.tile([128, 2, 512], F32, tag="sc",
                                            name=f"scb_{qb}_{t}_{g0}")
                            for j in range(gn):
                                kt = g0 + j
                                ksl = slice(kt * 128, (kt + 1) * 128)
                                nc.tensor.matmul(
                                    sca[:, j, :], kt_s[0:64, t, ksl],
                                    qt_b[0:64, t, :],
                                    start=True, stop=True, tile_position=(0, 0),
                                )
                                nc.tensor.matmul(
                                    scb[:, j, :], kt_s[64:128, t, ksl],
                                    qt_b[64:128, t, :],
                                    start=True, stop=True, tile_position=(64, 0),
                                )
                            ea = att.tile([128, 2, 512], F32R, tag="exp", bufs=6,
                                          name=f"ea_{qb}_{t}_{g0}")
                            eb = att.tile([128, 2, 512], F32R, tag="exp", bufs=6,
                                          name=f"eb_{qb}_{t}_{g0}")
                            nc.scalar.activation(ea[:, 0:gn, :], sca[:, 0:gn, :],
                                                 EXP, scale=0.125)
                            nc.scalar.activation(eb[:, 0:gn, :], scb[:, 0:gn, :],
                                                 EXP, scale=0.125)
                            ha, hb = 2 * t, 2 * t + 1
                            for j in range(gn):
                                kt = g0 + j
                                nc.tensor.matmul(
                                    ota[:], vext_s[:, kt, ha * VW:(ha + 1) * VW],
                                    ea[:, j, :],
                                    start=kt == 0, stop=kt == KT - 1,
                                )
                                nc.tensor.matmul(
                                    otb[:], vext_s[:, kt, hb * VW:(hb + 1) * VW],
                                    eb[:, j, :],
                                    start=kt == 0, stop=kt == KT - 1,
                                )
                        # normalize: rows 0:64 are O.T, row 64 the denominator
                        for nm, ot_ps, psl in (("a", ota, slice(0, 64)),
                                               ("b", otb, slice(64, 128))):
                            dn = att.tile([1, 512], F32, tag="dn", bufs=2,
                                          name=f"dn{nm}_{qb}_{t}")
                            nc.vector.tensor_copy(dn[:], ot_ps[64:65, :])
                            rd = att.tile([1, 512], F32, tag="rd", bufs=2,
                                          name=f"rd{nm}_{qb}_{t}")
                            nc.vector.reciprocal_approx_fast(rd[:], dn[:])
                            rb = att.tile([64, 512], F32, tag="rb", bufs=2,
                                          name=f"rb{nm}_{qb}_{t}")
                            nc.gpsimd.partition_broadcast(rb[:], rd[:])
                            nc.vector.tensor_tensor(
                                ot_b[psl, t, :], ot_ps[0:64, :], rb[:], MULT)

                    # ---- W_o stage for the 4 s-tiles of this q block ----
                    for si in range(4):
                        st = qb * 4 + si
                        ssl = slice(si * 128, (si + 1) * 128)
                        for dm in range(2):
                            ps = psacc.tile([128, 512], F32, tag="acc",
                                            name=f"pso_{st}_{dm}")
                            for t in range(NT):
                                nc.tensor.matmul(
                                    ps[:], ot_b[:, t, ssl],
                                    wot_s[:, t, dm * 512:(dm + 1) * 512],
                                    start=t == 0, stop=t == NT - 1,
                                )
                            ob = att.tile([128, 512], F32, tag="ob", bufs=3,
                                          name=f"ob_{st}_{dm}")
                            nc.vector.tensor_copy(ob[:], ps[:])
                            nc.sync.dma_start(
                                out[st * 128:(st + 1) * 128,
                                    dm * 512:(dm + 1) * 512],
                                ob[:])
    nc.compile()
    return nc


_NC_CACHE = []


def kernel(**inputs):
    query = np.asarray(inputs["query"], dtype=np.float32)
    key = np.asarray(inputs["key"], dtype=np.float32)
    value = np.asarray(inputs["value"], dtype=np.float32)
    w_q = np.asarray(inputs["W_q"], dtype=np.float32)
    w_k = np.asarray(inputs["W_k"], dtype=np.float32)
    w_v = np.asarray(inputs["W_v"], dtype=np.float32)
    w_o = np.asarray(inputs["W_o"], dtype=np.float32)

    in_maps = []
    for c in range(8):
        b, hg = c // 2, c % 2
        dsl = slice(hg * DP, (hg + 1) * DP)
        in_maps.append({
            "xqt": np.ascontiguousarray(query[b].T),
            "xkt": np.ascontiguousarray(key[b].T),
            "xvt": np.ascontiguousarray(value[b].T),
            "wqt": np.ascontiguousarray(w_q[dsl, :].T),
            "wkt": np.ascontiguousarray(w_k[dsl, :].T),
            "wvt": np.ascontiguousarray(w_v[dsl, :].T),
            "wot": np.ascontiguousarray(w_o[:, dsl].T),
        })

    if not _NC_CACHE:
        _NC_CACHE.append(build_nc())
    nc = _NC_CACHE[0]
    res = run_bass_kernel_spmd(nc, in_maps, core_ids=list(range(8)),
                               **_RUN_KWARGS)
    _LAST_RESULT.clear()
    _LAST_RESULT.append(res)
    parts = [r["out"] for r in res.results]
    full = np.empty((B, S, D), dtype=np.float32)
    for b in range(B):
        full[b] = parts[2 * b] + parts[2 * b + 1]
    return full


# revision 8
# speedup vs baseline: 1.5144x; 1.2639x over previous
"""MultiHeadAttention Trainium2 Bass kernel, 8-core (batch x head-group) sharded.

Reference computation (B=4, S=2048, D=1024, H=16, d_k=64):
    Q = query @ W_q.T ; K = key @ W_k.T ; V = value @ W_v.T
    per head: attn = softmax(Q K^T / 8) @ V
    out = concat_heads(attn) @ W_o.T

Sharding: core c handles batch b = c // 2 and head-group hg = c % 2 (8 heads,
a 512-wide slice of the model dim). Host pre-transposes activations/weights so
every on-device matmul contracts along partitions; core-pair partial outputs
(row-parallel W_o) are summed on the host during unsharding.

Per-core dataflow (all matmul inputs float32r):
    K.T[d', s] = (W_k.T slice).T @ x_k.T   (pattern B, d' on partitions)
    Q.T[d', s] likewise, projected on demand per 512-wide q-block
    V[s, d']   = (x_v.T).T @ W_v.T         (pattern A, natural layout)
    S.T[k, q]  = (K_h.T).T @ Q_h.T         (two heads row-packed, K=64)
    expS.T     = exp(S.T / 8)              (ACT, groups of 3 PSUM banks)
    O.T+denom  = [V_h | 1].T @ expS.T      (M=65, accumulated over k tiles)
    O.T norm   = O.T * (1/denom)           (DVE + gpsimd partition broadcast)
    out[s, :]  = O.T.T @ W_o.T slice       (partial; host adds core pairs)
"""
import sys

sys.path.insert(0, "/opt/trn_rl_repo")

import numpy as np

import concourse.bass as bass  # noqa: F401
import concourse.tile as tile
from concourse import bacc, mybir
from concourse.bass_utils import run_bass_kernel_spmd

F32R = mybir.dt.float32r
F32 = mybir.dt.float32
EXP = mybir.ActivationFunctionType.Exp
MULT = mybir.AluOpType.mult
DIV = mybir.AluOpType.divide

B, S, D = 4, 2048, 1024
H_PER_CORE = 8      # heads per core
DH = 64             # head dim
DP = 512            # per-core model-dim slice (8 heads x 64)
NT = 4              # d' tiles / head pairs per core
SB = 4              # 512-wide s/q blocks
KT = 16             # 128-wide k tiles
PKT = 8             # 128-wide contraction tiles for projections (D / 128)
VW = DH + 1         # V columns per head incl. ones column

_RUN_KWARGS = {}
_LAST_RESULT = []


def build_nc():
    nc = bacc.Bacc("TRN2", target_bir_lowering=False, debug=False)

    xqt = nc.dram_tensor("xqt", [D, S], F32R, kind="ExternalInput")
    xkt = nc.dram_tensor("xkt", [D, S], F32R, kind="ExternalInput")
    xvt = nc.dram_tensor("xvt", [D, S], F32R, kind="ExternalInput")
    wqt = nc.dram_tensor("wqt", [D, DP], F32R, kind="ExternalInput")
    wkt = nc.dram_tensor("wkt", [D, DP], F32R, kind="ExternalInput")
    wvt = nc.dram_tensor("wvt", [D, DP], F32R, kind="ExternalInput")
    wot = nc.dram_tensor("wot", [DP, D], F32R, kind="ExternalInput")
    out = nc.dram_tensor("out", [S, D], F32, kind="ExternalOutput")

    with tile.TileContext(nc) as tc:
        with tc.tile_pool(name="persist", bufs=1) as persist, \
             tc.tile_pool(name="psum", bufs=3, space="PSUM") as psum:

            # ---- persistent SBUF ----
            wq_s = persist.tile([128, PKT, DP], F32R)
            wot_s = persist.tile([128, NT, D], F32R)
            kt_s = persist.tile([128, NT, S], F32R)          # K.T
            vext_s = persist.tile([128, KT, H_PER_CORE * VW], F32R)  # [V_h | 1]

            nc.gpsimd.dma_start(wq_s[:], wqt.rearrange("(t p) m -> p t m", p=128))
            nc.gpsimd.dma_start(wot_s[:], wot.rearrange("(t p) m -> p t m", p=128))
            # ones columns for the denominator rows (V part is written below)
            ones_f = persist.tile([128, KT, H_PER_CORE], F32)
            nc.vector.memset(ones_f[:], 1.0)
            nc.vector.tensor_copy(
                vext_s[:].rearrange("p k (h c) -> p k h c", c=VW)[:, :, :, DH:DH + 1],
                ones_f[:, :, :, None],
            )

            # ============ phase 1+2: K.T and V projections ============
            with tc.tile_pool(name="proj", bufs=2) as proj:
                wk_s = proj.tile([128, PKT, DP], F32R, bufs=1)
                wv_s = proj.tile([128, PKT, DP], F32R, bufs=1)
                nc.gpsimd.dma_start(wk_s[:], wkt.rearrange("(t p) m -> p t m", p=128))
                nc.gpsimd.dma_start(wv_s[:], wvt.rearrange("(t p) m -> p t m", p=128))

                # V projection into [V_h | 1] layout (2 s-tiles per DMA)
                for sg in range(KT // 2):
                    xv_b = proj.tile([128, PKT, 256], F32R, tag="xv",
                                     name=f"xv_{sg}")
                    nc.sync.dma_start(
                        xv_b[:],
                        xvt[:, sg * 256:(sg + 1) * 256].rearrange(
                            "(t p) s -> p t s", p=128),
                    )
                    for half in range(2):
                        st = sg * 2 + half
                        ps = psum.tile([128, 512], F32, tag="sc",
                                        name=f"psv_{st}")
                        for kt in range(PKT):
                            nc.tensor.matmul(
                                ps[:], xv_b[:, kt, half * 128:(half + 1) * 128],
                                wv_s[:, kt, :],
                                start=kt == 0, stop=kt == PKT - 1,
                            )
                        nc.vector.tensor_copy(
                            vext_s[:, st, :].rearrange(
                                "p (h c) -> p h c", c=VW)[:, :, 0:DH],
                            ps[:].rearrange("p (h c) -> p h c", c=DH),
                        )

                # K.T projection: kt_s[:, t, sb*512:] = (wk col-block).T @ xk.T
                for sb in range(SB):
                    xk_b = proj.tile([128, PKT, 512], F32R, tag="xk",
                                     name=f"xk_{sb}")
                    nc.sync.dma_start(
                        xk_b[:],
                        xkt[:, sb * 512:(sb + 1) * 512].rearrange(
                            "(t p) s -> p t s", p=128),
                    )
                    for t in range(NT):
                        ps = psum.tile([128, 512], F32, tag="sc",
                                        name=f"psk_{sb}_{t}")
                        for kt in range(PKT):
                            nc.tensor.matmul(
                                ps[:],
                                wk_s[:, kt, t * 128:(t + 1) * 128],
                                xk_b[:, kt, :],
                                start=kt == 0, stop=kt == PKT - 1,
                            )
                        nc.vector.tensor_copy(
                            kt_s[:, t, sb * 512:(sb + 1) * 512], ps[:])
            # ============ phase 3: attention + W_o, per 512-wide q block ==========
            with tc.tile_pool(name="att", bufs=2) as att:
                for qb in range(SB):
                    qsl = slice(qb * 512, (qb + 1) * 512)

                    # Q.T for this q block, all 4 d' tiles
                    xq_b = att.tile([128, PKT, 512], F32R, tag="xq", bufs=1,
                                    name=f"xq_{qb}")
                    nc.sync.dma_start(
                        xq_b[:],
                        xqt[:, qsl].rearrange("(t p) s -> p t s", p=128),
                    )
                    qt_b = att.tile([128, NT, 512], F32R, tag="qt",
                                    name=f"qt_{qb}")
                    for t in range(NT):
                        ps = psum.tile([128, 512], F32, tag="sc",
                                        name=f"psq_{qb}_{t}")
                        for kt in range(PKT):
                            nc.tensor.matmul(
                                ps[:],
                                wq_s[:, kt, t * 128:(t + 1) * 128],
                                xq_b[:, kt, :],
                                start=kt == 0, stop=kt == PKT - 1,
                            )
                        nc.vector.tensor_copy(qt_b[:, t, :], ps[:])

                    ot_b = att.tile([128, NT, 512], F32R, tag="ot",
                                    name=f"ot_{qb}")
                    for t in range(NT):
                        # two heads: A on partitions 0:64, B on 64:128
                        ota = psum.tile([65, 512], F32, tag="ot", bufs=2,
                                        name=f"ota_{qb}_{t}")
                        otb = psum.tile([65, 512], F32, tag="ot", bufs=2,
                                        name=f"otb_{qb}_{t}")
                        ha, hb = 2 * t, 2 * t + 1
                        for kt in range(KT):
                            ksl = slice(kt * 128, (kt + 1) * 128)
                            sc = psum.tile([128, 2, 512], F32, tag="sc",
                                           name=f"sc_{qb}_{t}_{kt}")
                            nc.tensor.matmul(
                                sc[:, 0, :], kt_s[0:64, t, ksl],
                                qt_b[0:64, t, :],
                                start=True, stop=True, tile_position=(0, 0),
                            )
                            nc.tensor.matmul(
                                sc[:, 1, :], kt_s[64:128, t, ksl],
                                qt_b[64:128, t, :],
                                start=True, stop=True, tile_position=(64, 0),
                            )
                            e = att.tile([128, 2, 512], F32R, tag="exp", bufs=6,
                                         name=f"e_{qb}_{t}_{kt}")
                            nc.scalar.activation(e[:], sc[:], EXP, scale=0.125)
                            nc.tensor.matmul(
                                ota[:], vext_s[:, kt, ha * VW:(ha + 1) * VW],
                                e[:, 0, :],
                                start=kt == 0, stop=kt == KT - 1,
                            )
                            nc.tensor.matmul(
                                otb[:], vext_s[:, kt, hb * VW:(hb + 1) * VW],
                                e[:, 1, :],
                                start=kt == 0, stop=kt == KT - 1,
                            )
                        # normalize: rows 0:64 are O.T, row 64 the denominator
                        for nm, ot_ps, psl in (("a", ota, slice(0, 64)),
                                               ("b", otb, slice(64, 128))):
                            dn = att.tile([1, 512], F32, tag="dn", bufs=2,
                                          name=f"dn{nm}_{qb}_{t}")
                            nc.vector.tensor_copy(dn[:], ot_ps[64:65, :])
                            rd = att.tile([1, 512], F32, tag="rd", bufs=2,
                                          name=f"rd{nm}_{qb}_{t}")
                            nc.vector.reciprocal_approx_fast(rd[:], dn[:])
                            rb = att.tile([64, 512], F32, tag="rb", bufs=2,
                                          name=f"rb{nm}_{qb}_{t}")
                            nc.gpsimd.partition_broadcast(rb[:], rd[:])
                            nc.vector.tensor_tensor(
                                ot_b[psl, t, :], ot_ps[0:64, :], rb[:], MULT)

                    # ---- W_o stage for the 4 s-tiles of this q block ----
                    for si in range(4):
                        st = qb * 4 + si
                        ssl = slice(si * 128, (si + 1) * 128)
                        for dm in range(2):
                            ps = psum.tile([128, 512], F32, tag="sc",
                                            name=f"pso_{st}_{dm}")
                            for t in range(NT):
                                nc.tensor.matmul(
                                    ps[:], ot_b[:, t, ssl],
                                    wot_s[:, t, dm * 512:(dm + 1) * 512],
                                    start=t == 0, stop=t == NT - 1,
                                )
                            ob = att.tile([128, 512], F32, tag="ob", bufs=3,
                                          name=f"ob_{st}_{dm}")
                            nc.vector.tensor_copy(ob[:], ps[:])
                            nc.sync.dma_start(
                                out[st * 128:(st + 1) * 128,
                                    dm * 512:(dm + 1) * 512],
                                ob[:])
    nc.compile()
    return nc


_NC_CACHE = []


def kernel(**inputs):
    query = np.asarray(inputs["query"], dtype=np.float32)
    key = np.asarray(inputs["key"], dtype=np.float32)
    value = np.asarray(inputs["value"], dtype=np.float32)
    w_q = np.asarray(inputs["W_q"], dtype=np.float32)
    w_k = np.asarray(inputs["W_k"], dtype=np.float32)
    w_v = np.asarray(inputs["W_v"], dtype=np.float32)
    w_o = np.asarray(inputs["W_o"], dtype=np.float32)

    in_maps = []
    for c in range(8):
        b, hg = c // 2, c % 2
        dsl = slice(hg * DP, (hg + 1) * DP)
        in_maps.append({
            "xqt": np.ascontiguousarray(query[b].T),
            "xkt": np.ascontiguousarray(key[b].T),
            "xvt": np.ascontiguousarray(value[b].T),
            "wqt": np.ascontiguousarray(w_q[dsl, :].T),
            "wkt": np.ascontiguousarray(w_k[dsl, :].T),
            "wvt": np.ascontiguousarray(w_v[dsl, :].T),
            "wot": np.ascontiguousarray(w_o[:, dsl].T),
        })

    if not _NC_CACHE:
        _NC_CACHE.append(build_nc())
    nc = _NC_CACHE[0]
    res = run_bass_kernel_spmd(nc, in_maps, core_ids=list(range(8)),
                               **_RUN_KWARGS)
    _LAST_RESULT.clear()
    _LAST_RESULT.append(res)
    parts = [r["out"] for r in res.results]
    full = np.empty((B, S, D), dtype=np.float32)
    for b in range(B):
        full[b] = parts[2 * b] + parts[2 * b + 1]
    return full
